# revision 17
# baseline (speedup 1.0000x reference)
"""Multi-head attention (B=4, S=1024, D=1024, H=16) on 8 Trainium2 NeuronCores.

Sharding (exactly balanced): each core owns TWO heads (a head-pair) of ALL
FOUR batches. Per-core attention work is then 2*sum_b ceil(valid_len_b/128)
key-tile units regardless of how the valid_lens are distributed -- no core
ever computes a masked-out key tile, unlike batch-paired layouts that pad
every core to a shared compile-time tile count. Batches are processed as 4
slots sorted by descending tile count; the program is cached per sorted
tile-count tuple. Each core emits one [S, D] partial per batch (its 2
heads through the row-sharded W_o); the host sums the 8 partials per batch
and adds b_o.

All matmul operands are fp16 (PSUM accumulation stays fp32).

Queue discipline: the Sync (SP) hardware DGE queue executes descriptors in
order, so a descriptor that waits on a ring-buffer dependency would block
everything behind it. Only dependency-free head loads and the output
stores go on Sync; all ring-reusing x loads and the tiny softmax
denominator spread/gather DMAs go through the otherwise-idle GPSIMD
software DGE. The denominator gather also casts fp32r->fp16 (only the
GPSIMD DGE can cast) so the reciprocal-broadcast matmul runs at fp16
speed.

Schedule: V projections are interleaved into the NEXT slot's score loop
(one key-tile chunk of V per score tile): the ACT engine's exp drain
(1.1us per [128,1024] tile) is slower than the PE score matmuls, so the
V-proj matmuls soak up the PE slack instead of idling. Output projection
blocks run between attention phases; output staging is [128, 8, 1024] per
parity so the two per-batch stores never stall the next block's copies.
Every finish_norm is deferred past the next block of PE work so its
gather->reciprocal->scatter chain stays off the in-order PE queue.

Device layout tricks:
- Q/K are produced transposed ([dh, s]) straight out of the projection
  matmuls, so attention scores come out as scoresT[sk, sq] with key
  positions on partitions. The padding mask is a per-partition bias on the
  fused exp activation (exp(0.125*s - 1e6) == 0); the per-slot bias
  columns are packed into one [128, sum(cts)] tensor (single descriptor).
- V is produced in [sk, dh] layout with an extra "ones" column per head:
  one matmul accumulation yields the attention numerator (partitions
  0..63) and the softmax denominator (partition 64) together.
- 1/denominator is broadcast across each head's 64 rows with a tiny
  selector matmul, then folded in with one elementwise multiply
  (fp32 numerators in, fp16 normalized out).
"""

import numpy as np

import concourse.bacc as bacc
import concourse.tile as tile
import concourse.mybir as mybir
from concourse.bass_utils import run_bass_kernel_spmd

F32 = mybir.dt.float32
F32R = mybir.dt.float32r
F16 = mybir.dt.float16
EXP = mybir.ActivationFunctionType.Exp

B, S, D, H = 4, 1024, 1024, 16
DH = D // H            # 64
G2 = 2                 # heads per core
QC = G2 * DH           # 128 projection cols per core
NEG = -1000000.0
P = 128
NDC = D // P           # 8 contraction chunks
NSL = 4                # slots per core (one per batch)
VW = DH + 1            # 65: per-head V slot width (64 V cols + ones col)
WU = 26                # warmup matmuls (PE p-state ramp during startup DMA)


def build(cts, with_bq, with_bk, with_bv):
    """Build the SPMD program. cts = per-slot key-tile counts, desc order."""
    nc = bacc.Bacc(None, target_bir_lowering=False, debug=False)

    sks = [ct * P for ct in cts]
    kgs = [[(s0, min(512, sk - s0)) for s0 in range(0, sk, 512)] for sk in sks]
    # parity tag sizes: slots s and s+2 share SBUF tiles sized for s (desc)
    pcts = [cts[0], cts[1]]
    pkgs = [kgs[0], kgs[1]]
    CTS = sum(cts)
    moff = [sum(cts[:s]) for s in range(NSL)]

    xq_d = [nc.dram_tensor(f"xq{s}", [P, NDC, S], F16, kind="ExternalInput")
            for s in range(NSL)]
    xk_d = [[nc.dram_tensor(f"xk{s}{g}", [P, NDC, w], F16, kind="ExternalInput")
             for g, (s0, w) in enumerate(kgs[s])] for s in range(NSL)]
    xv_d = [[nc.dram_tensor(f"xv{s}{g}", [P, NDC, w], F16, kind="ExternalInput")
             for g, (s0, w) in enumerate(kgs[s])] for s in range(NSL)]
    wq = nc.dram_tensor("wq", [P, NDC, QC], F16, kind="ExternalInput")
    wk = nc.dram_tensor("wk", [P, NDC, QC], F16, kind="ExternalInput")
    wv = nc.dram_tensor("wv", [P, NDC, QC], F16, kind="ExternalInput")
    wo = nc.dram_tensor("wo", [P, D], F16, kind="ExternalInput")
    mkb = nc.dram_tensor("mkb", [P, CTS], F32, kind="ExternalInput")  # 0/-1e6
    esel = nc.dram_tensor("esel", [2, P], F16, kind="ExternalInput")
    bq = nc.dram_tensor("bq", [QC], F32, kind="ExternalInput")
    bk = nc.dram_tensor("bk", [QC], F32, kind="ExternalInput")
    bv = nc.dram_tensor("bv", [QC], F16, kind="ExternalInput")
    outs = [nc.dram_tensor(f"out{s}", [S, D], F16, kind="ExternalOutput")
            for s in range(NSL)]

    with tile.TileContext(nc) as tc:
        with tc.tile_pool(name="persist", bufs=1) as persist, \
             tc.tile_pool(name="cst", bufs=1) as cst:
            # parity-shared persistent tiles
            qts = [persist.tile([P, S], F16, tag=f"qt{p}", name=f"qt{p}")
                   for p in range(2)]                          # QT[dh, sq]
            ktg = [[persist.tile([P, w], F16, tag=f"kt{p}_{g}", name=f"kt{p}_{g}")
                    for g, (s0, w) in enumerate(pkgs[p])] for p in range(2)]
            vps = [persist.tile([P, pcts[p], G2, VW], F16, tag=f"vp{p}",
                                name=f"vp{p}") for p in range(2)]
            mgs = [persist.tile([P, S], F16, tag=f"mg{p}", name=f"mg{p}")
                   for p in range(2)]                          # normalized
            obs = [persist.tile([P, 8, D], F16, tag=f"ob{p}", name=f"ob{p}")
                   for p in range(2)]                          # out staging

            mbs = cst.tile([P, CTS], F32, tag="mb", name="mb")
            es = cst.tile([P, P], F16, tag="es")
            if with_bq:
                bq_sb = cst.tile([P, 1], F32, tag="bq")
                nc.sync.dma_start(out=bq_sb[:], in_=bq.rearrange("(t p) -> p t", p=P))
            if with_bk:
                bk_sb = cst.tile([P, 1], F32, tag="bk")
                nc.sync.dma_start(out=bk_sb[:], in_=bk.rearrange("(t p) -> p t", p=P))
            if with_bv:
                bv_sb = cst.tile([1, QC], F16, tag="bv")
                nc.sync.dma_start(out=bv_sb[:], in_=bv[None, :])
                ones1f = cst.tile([1, P], F32, tag="ones1f")
                nc.vector.memset(ones1f[:], 1.0)
                ones1 = cst.tile([1, P], F16, tag="ones1")
                nc.vector.tensor_copy(ones1[:], ones1f[:])

            # ACT exp-table preload during the startup DMA wait.
            wtb = cst.tile([1, 16], F32, tag="wtb")
            wtb0 = cst.tile([1, 1], F32, tag="wtb0")
            nc.vector.memset(wtb[:], 0.0)
            nc.vector.memset(wtb0[:], 0.0)
            nc.scalar.activation(wtb[:], wtb[:], EXP, bias=wtb0[:], scale=1.0)

            scr = cst.tile([P, 640], F16, tag="scr")
            nc.vector.memset(scr[:], 0.001)

            # V slots: ones column at position DH of every head slot.
            onesw = cst.tile([P, max(cts)], F32, tag="onesw")
            nc.vector.memset(onesw[:], 1.0)
            for p in range(2):
                for h in range(G2):
                    nc.vector.tensor_copy(vps[p][:, :, h, DH], onesw[:, :pcts[p]])

            wts_cm = tc.tile_pool(name="wts", bufs=1)
            wts = wts_cm.__enter__()
            xs_cm = tc.tile_pool(name="xs", bufs=1)
            xs = xs_cm.__enter__()

            # parity-shared x tiles (slot s loads into parity s%2's tile)
            xq_sb = [[xs.tile([P, NDC // 2, S], F16, tag=f"xq{p}_{i}",
                              name=f"xq{p}_{i}") for i in range(2)]
                     for p in range(2)]
            xk_sb = [[xs.tile([P, NDC, w], F16, tag=f"xk{p}_{g}",
                              name=f"xk{p}_{g}") for g, (s0, w) in enumerate(pkgs[p])]
                     for p in range(2)]
            xv_sb = [[xs.tile([P, NDC, w], F16, tag=f"xv{p}_{g}",
                              name=f"xv{p}_{g}") for g, (s0, w) in enumerate(pkgs[p])]
                     for p in range(2)]

            def xqload(s, eng):
                for i in range(2):
                    eng.dma_start(
                        out=xq_sb[s % 2][i][:],
                        in_=xq_d[s][:, i * (NDC // 2):(i + 1) * (NDC // 2), :])

            def kvload(drams, sb, s, eng):
                for g, d in enumerate(drams[s]):
                    w = d.shape[2]
                    eng.dma_start(out=sb[s % 2][g][:, :, :w], in_=d[:, :, :])

            def xat(p, c):
                return xq_sb[p][c // (NDC // 2)][:, c % (NDC // 2), :]

            # PSUM budget (8 banks): psS = 2 x [P,1024] (4 banks) for the
            # scores/exp pipeline; ps5 = 4 x [P,512] ring shared by warmup,
            # Q/K/V projections, attnV accumulators, outproj and the
            # recip-broadcast.
            with tc.tile_pool(name="dpool", bufs=2) as dpool, \
                 tc.tile_pool(name="psS", bufs=2, space="PSUM") as psS, \
                 tc.tile_pool(name="ps5", bufs=4, space="PSUM") as ps5:
                # dummy matmuls keep the PE array clocking up until the
                # first x wire lands (HAM re-throttles after ~3.4us idle).
                pswu = ps5.tile([P, 512], F32, tag="ps5", name="pswu")
                for i in range(WU):
                    nc.tensor.matmul(pswu[:], scr[:, 0:128], scr[:, 128:640],
                                     start=(i == 0), stop=(i == WU - 1))

                # head loads, highest-urgency first. Transfers serialize
                # per DGE queue at ~300GB/s, so the first xq rides the
                # (idle-until-first-exp) Scalar hardware DGE queue in
                # parallel with the Sync stream.
                wq_sb = wts.tile([P, NDC, QC], F16, tag="wq")
                nc.sync.dma_start(out=wq_sb[:], in_=wq[:, :, :])
                wk_sb = wts.tile([P, NDC, QC], F16, tag="wk")
                nc.sync.dma_start(out=wk_sb[:], in_=wk[:, :, :])
                xqload(0, nc.scalar)
                kvload(xk_d, xk_sb, 0, nc.sync)
                nc.sync.dma_start(out=mbs[:], in_=mkb[:, :])
                nc.sync.dma_start(out=es[64:66, :], in_=esel[:, :])
                wv_sb = wts.tile([P, NDC, QC], F16, tag="wv")
                nc.sync.dma_start(out=wv_sb[:], in_=wv[:, :, :])
                kvload(xv_d, xv_sb, 0, nc.sync)
                xqload(1, nc.sync)
                kvload(xk_d, xk_sb, 1, nc.sync)
                kvload(xv_d, xv_sb, 1, nc.sync)
                wo_sb = wts.tile([P, D], F16, tag="wo")
                nc.sync.dma_start(out=wo_sb[:], in_=wo[:, :])

                def dummy():
                    # clock-keeper filler: burns PE slack under the ACT exp
                    # drain so HAM never sees an idle window and throttles
                    psd = ps5.tile([P, 512], F32, tag="ps5", name="psd")
                    for i in range(2):
                        nc.tensor.matmul(psd[:], scr[:, 0:128], scr[:, 128:640],
                                         start=(i == 0), stop=(i == 1))

                def kproj_emit(s, g):
                    p = s % 2
                    s0, w = kgs[s][g]
                    psk = ps5.tile([P, 512], F32, tag="ps5", name=f"psK_{g}_{s}")
                    for c in range(NDC):
                        nc.tensor.matmul(
                            psk[:, :w], wk_sb[:, c, :], xk_sb[p][g][:, c, :w],
                            start=(c == 0), stop=(c == NDC - 1))
                    if with_bk:
                        nc.vector.tensor_scalar_add(
                            ktg[p][g][:, :w], psk[:, :w], bk_sb[:, 0:1])
                    else:
                        nc.vector.tensor_copy(ktg[p][g][:, :w], psk[:, :w])

                def vchunk(s, st):
                    # one key-tile of V projection (filler for PE slack)
                    p = s % 2
                    psv = ps5.tile([P, 512], F32, tag="ps5", name=f"psv{s}_{st}")
                    g, o = st // 4, (st % 4) * P
                    for c in range(NDC):
                        nc.tensor.matmul(
                            psv[:, 0:QC], xv_sb[p][g][:, c, o:o + P],
                            wv_sb[:, c, :], start=(c == 0),
                            stop=(c == NDC - 1 and not with_bv))
                    if with_bv:
                        nc.tensor.matmul(psv[:, 0:QC], ones1[:], bv_sb[:],
                                         start=False, stop=True)
                    nc.vector.tensor_copy(
                        vps[p][:, st, :, 0:DH],
                        psv[:, 0:QC].rearrange("p (g d) -> p g d", g=G2))

                def pair_emit(s, fillers):
                    # Q projection, then per K group: K proj + score tiles,
                    # with the fused exp(0.125*s + maskbias) on ACT. One
                    # filler (V chunk of the previous slot) is emitted after
                    # each score tile to fill PE slack under the exp drain;
                    # leftovers run after the loop.
                    p = s % 2
                    skt = cts[s]
                    fillers = list(fillers)
                    # split-pass Q: q-half0 consumes xq chunk-half 0 first so
                    # the projection starts before the second xq DMA half
                    # lands. q-half1's accumulation is ROTATED to start at
                    # c=4 so the first instruction touching the late xq half
                    # is an accumulation-group leader (gets the full DMA
                    # wait), never a mid-group instruction.
                    psqs = [ps5.tile([P, 512], F32, tag="ps5",
                                     name=f"psQ_{s}_{half}")
                            for half in range(2)]
                    corder = [list(range(NDC)),
                              list(range(4, NDC)) + list(range(4))]
                    for half, cpass in ((0, 0), (1, 0), (0, 1), (1, 1)):
                        qsl = slice(half * 512, (half + 1) * 512)
                        cs = corder[half][cpass * 4:cpass * 4 + 4]
                        for c in cs:
                            nc.tensor.matmul(
                                psqs[half][:], wq_sb[:, c, :],
                                xat(p, c)[:, qsl],
                                start=(c == corder[half][0]),
                                stop=(c == corder[half][-1]))
                        if s == 0 and half == 0 and cpass == 0:
                            # bridge the wait for the second xq half
                            dummy()
                            dummy()
                    for half in range(2):
                        qsl = slice(half * 512, (half + 1) * 512)
                        if with_bq:
                            nc.vector.tensor_scalar_add(
                                qts[p][:, qsl], psqs[half][:], bq_sb[:, 0:1])
                        else:
                            nc.vector.tensor_copy(qts[p][:, qsl], psqs[half][:])
                    pte = dpool.tile([P, pcts[p], S], F16, tag=f"pe{p}",
                                     name=f"pe{s}", bufs=1)
                    pto = dpool.tile([P, pcts[p], S], F16, tag=f"po{p}",
                                     name=f"po{s}", bufs=1)
                    nfill = 0
                    for g in range(len(kgs[s])):
                        kproj_emit(s, g)
                        for st in range(4 * g, min(4 * (g + 1), skt)):
                            kt = ktg[p][g][:, (st - 4 * g) * P:(st - 4 * g + 1) * P]
                            pse = psS.tile([P, S], F32, tag="psS",
                                           name=f"psSe_{s}_{st}")
                            pso = psS.tile([P, S], F32, tag="psS",
                                           name=f"psSo_{s}_{st}")
                            for half in range(2):
                                qsl = slice(half * 512, (half + 1) * 512)
                                nc.tensor.matmul(pse[:, qsl], kt[0:64, :],
                                                 qts[p][0:64, qsl],
                                                 start=True, stop=True)
                                nc.tensor.matmul(pso[:, qsl], kt[64:128, :],
                                                 qts[p][64:128, qsl],
                                                 start=True, stop=True)
                            mb = mbs[:, moff[s] + st:moff[s] + st + 1]
                            nc.scalar.activation(pte[:, st, :], pse[:], EXP,
                                                 bias=mb, scale=0.125)
                            nc.scalar.activation(pto[:, st, :], pso[:], EXP,
                                                 bias=mb, scale=0.125)
                            if st > 0 and nfill < len(fillers):
                                fillers[nfill]()
                                nfill += 1
                    for f in fillers[nfill:]:
                        f()
                    return pte, pto

                def attnv_emit(s, pte, pto):
                    # Heavy attnV matmuls + the denominator recip pipeline.
                    # The PE-side recip BROADCAST + final multiply are
                    # deferred (returned as a closure) so later PE work
                    # covers the recip chain latency.
                    p = s % 2
                    skt = cts[s]
                    parts = []
                    for half in range(2):
                        qsl = slice(half * 512, (half + 1) * 512)
                        mgn = dpool.tile([P, 512], F16, tag="mgn", bufs=4,
                                         name=f"mgn_{s}_{half}")
                        dst_t = dpool.tile([P, 2, 512], F32, tag="dstp")
                        rsg_t = dpool.tile([P, 128], F32, tag="rsgp")
                        rcp_t = dpool.tile([P, 128], F16, tag="rcpp")
                        rst_t = dpool.tile([P, 512], F16, tag="rstp", bufs=4)
                        for hi, pt in enumerate((pte, pto)):
                            nv = ps5.tile([P, 512], F32, tag="ps5",
                                          name=f"nv_{s}_{half}_{hi}")
                            for st in range(skt):
                                nc.tensor.matmul(nv[0:DH + 1, :],
                                                 vps[p][:, st, hi, :],
                                                 pt[:, st, qsl],
                                                 start=(st == 0),
                                                 stop=(st == skt - 1))
                            # numerator staging: Vector early slots, Scalar
                            # late slots (ACT exp pressure tapers off)
                            if s >= 2:
                                nc.scalar.copy(mgn[64 * hi:64 * hi + 64, :],
                                               nv[0:64, :])
                            else:
                                nc.vector.tensor_copy(
                                    mgn[64 * hi:64 * hi + 64, :], nv[0:64, :])
                            nc.vector.tensor_copy(dst_t[64:65, hi, :],
                                                  nv[64:65, :])
                            # spread the denominator row over 4 partitions
                            # for a fast lane-parallel reciprocal
                            nc.gpsimd.dma_start(
                                out=rsg_t[64 + 4 * hi:68 + 4 * hi, :],
                                in_=dst_t[64:65, hi, :])
                        with nc.allow_low_precision("softmax denom recip fp16"):
                            nc.vector.reciprocal(rcp_t[64:72, :],
                                                 rsg_t[64:72, :])
                        nc.gpsimd.dma_start(out=rst_t[64:66, :],
                                            in_=rcp_t[64:72, :])
                        parts.append((qsl, mgn, rst_t))

                    def finish_norm():
                        # broadcast matmuls go through psS (idle once the
                        # slot's scores are done) to stay off the busy ps5
                        # ring
                        prt = psS.tile([P, S], F32, tag="psS", name=f"pr_{s}")
                        for hi, (qsl, mgn, rst_t) in enumerate(parts):
                            pr = prt[:, hi * 512:(hi + 1) * 512]
                            nc.tensor.matmul(pr, es[64:66, :],
                                             rst_t[64:66, :],
                                             start=True, stop=True)
                            nc.vector.tensor_mul(mgs[p][:, qsl], mgn[:], pr)
                    return finish_norm

                def outproj_emit(s):
                    # [128q, 1024] blocks; single 128-dim contraction chunk.
                    # PSUM alternates between the ps5 ring and the (now
                    # score-idle) psS ring per q-tile, doubling the
                    # matmul-ahead-of-copies depth.
                    p = s % 2
                    for qt_i in range(8):
                        sqsl = slice(qt_i * P, (qt_i + 1) * P)
                        if qt_i % 2 == 0:
                            halves = [ps5.tile([P, 512], F32, tag="ps5",
                                               name=f"psO_{s}_{qt_i}_{oh}")[:]
                                      for oh in range(2)]
                        else:
                            pq = psS.tile([P, S], F32, tag="psS",
                                          name=f"psO_{s}_{qt_i}")
                            halves = [pq[:, 0:512], pq[:, 512:1024]]
                        for oh in range(2):
                            osl = slice(oh * 512, (oh + 1) * 512)
                            nc.tensor.matmul(halves[oh], mgs[p][:, sqsl],
                                             wo_sb[:, osl],
                                             start=True, stop=True)
                        for oh in range(2):
                            osl = slice(oh * 512, (oh + 1) * 512)
                            if oh == 0:
                                nc.vector.tensor_copy(obs[p][:, qt_i, osl],
                                                      halves[oh])
                            else:
                                nc.scalar.copy(obs[p][:, qt_i, osl],
                                               halves[oh])
                        if qt_i % 4 == 3:
                            hsl = slice((qt_i - 3) * P, (qt_i + 1) * P)
                            # alternate stores across the two HWDGE queues
                            eng = nc.sync if qt_i == 3 else nc.scalar
                            eng.dma_start(
                                out=outs[s][hsl, :].rearrange(
                                    "(t p) d -> p t d", p=P),
                                in_=obs[p][:, qt_i - 3:qt_i + 1, :])

                # schedule: see module docstring. V_s chunks fill slot s's
                # own score loop; leading dummies cover the window before
                # xv_s lands. Ring-reuse x loads stay on the Sync queue,
                # positioned so each wait clears before the next is due.
                vth = [[(lambda s=s, st=st: vchunk(s, st))
                        for st in range(cts[s])] for s in range(NSL)]
                pt0 = pair_emit(0, [dummy, dummy] + vth[0])
                xqload(2, nc.sync)
                kvload(xk_d, xk_sb, 2, nc.sync)
                pt1 = pair_emit(1, [dummy] + vth[1])
                kvload(xv_d, xv_sb, 2, nc.sync)
                fn0 = attnv_emit(0, *pt0)
                xqload(3, nc.sync)
                kvload(xk_d, xk_sb, 3, nc.sync)
                pt2 = pair_emit(2, vth[2])
                fn0()
                kvload(xv_d, xv_sb, 3, nc.sync)
                fn1 = attnv_emit(1, *pt1)
                outproj_emit(0)
                pt3 = pair_emit(3, vth[3])
                fn1()
                fn2 = attnv_emit(2, *pt2)
                outproj_emit(1)
                fn2()
                fn3 = attnv_emit(3, *pt3)
                outproj_emit(2)
                fn3()
                outproj_emit(3)

            xs_cm.__exit__(None, None, None)
            wts_cm.__exit__(None, None, None)

    nc.finalize()
    return nc


_CACHE = {}


def _swz(a, width):
    """[D, width] -> [128, NDC, width] chunk-preswizzled contiguous array."""
    return np.ascontiguousarray(
        a.reshape(NDC, P, width).transpose(1, 0, 2)).astype(np.float16)


def kernel(**inputs):
    queries = np.asarray(inputs["queries"], np.float32)
    keys = np.asarray(inputs["keys"], np.float32)
    values = np.asarray(inputs["values"], np.float32)
    valid_lens = np.asarray(inputs["valid_lens"], np.int32)
    W_q = np.asarray(inputs["W_q"], np.float32)
    W_k = np.asarray(inputs["W_k"], np.float32)
    W_v = np.asarray(inputs["W_v"], np.float32)
    W_o = np.asarray(inputs["W_o"], np.float32)
    b_q = np.asarray(inputs["b_q"], np.float32)
    b_k = np.asarray(inputs["b_k"], np.float32)
    b_v = np.asarray(inputs["b_v"], np.float32)
    b_o = np.asarray(inputs["b_o"], np.float32)

    skt_b = np.maximum(1, np.minimum(8, -(-valid_lens // P)))
    order = [int(x) for x in np.argsort(-skt_b, kind="stable")]
    cts = tuple(int(skt_b[b]) for b in order)
    with_bq, with_bk, with_bv = bool(b_q.any()), bool(b_k.any()), bool(b_v.any())

    key = (cts, with_bq, with_bk, with_bv)
    if key not in _CACHE:
        _CACHE[key] = build(cts, with_bq, with_bk, with_bv)
    nc = _CACHE[key]

    esel = np.zeros((2, P), np.float16)
    esel[0, 0:DH] = 1.0
    esel[1, DH:2 * DH] = 1.0

    sks = [int(c) * P for c in cts]
    kgs = [[(s0, min(512, sk - s0)) for s0 in range(0, sk, 512)] for sk in sks]
    # packed per-slot mask bias columns: mkb[p, off_s + st] for key t*128+p
    mkb = np.concatenate(
        [(np.where(np.arange(sks[s]) < valid_lens[order[s]], 0.0, NEG)
          .reshape(cts[s], P).T) for s in range(NSL)],
        axis=1).astype(np.float32)

    # per-batch x swizzles computed once, shared across all 8 core maps
    shared = {"mkb": np.ascontiguousarray(mkb), "esel": esel}
    for s in range(NSL):
        b = order[s]
        shared[f"xq{s}"] = _swz(queries[b].T, S)
        for gi, (s0, w) in enumerate(kgs[s]):
            shared[f"xk{s}{gi}"] = _swz(
                np.ascontiguousarray(keys[b].T[:, s0:s0 + w]), w)
            shared[f"xv{s}{gi}"] = _swz(
                np.ascontiguousarray(values[b].T[:, s0:s0 + w]), w)

    in_maps = []
    for c in range(8):
        gsl = slice(c * QC, (c + 1) * QC)
        im = {
            "wq": _swz(W_q.T[:, gsl], QC),
            "wk": _swz(W_k.T[:, gsl], QC),
            "wv": _swz(W_v.T[:, gsl], QC),
            "wo": np.ascontiguousarray(W_o.T[gsl, :]).astype(np.float16),
            "bq": np.ascontiguousarray(b_q[gsl]),
            "bk": np.ascontiguousarray(b_k[gsl]),
            "bv": np.ascontiguousarray(b_v[gsl]).astype(np.float16),
        }
        im.update(shared)
        in_maps.append(im)

    res = run_bass_kernel_spmd(nc, in_maps, list(range(8)))
    final = np.zeros((B, S, D), np.float32)
    for c in range(8):
        for s in range(NSL):
            final[order[s]] += res.results[c][f"out{s}"].astype(np.float32)
    final += b_o
    return final
